# Initial kernel scaffold
#
"""GPT2 block kernel for 8 TRN2 NeuronCores.

Sharding: rows (batch*seq) split 8 ways -> 512 rows/core. Each core
redundantly computes K,V for its batch (4 cores share a batch), then
causal attention for its own 512 query rows against all 2048 keys
(mask as data), then proj/LN2/MLP for its own rows only.

Everything is computed in "transposed" layout (feature dim on
partitions, token dim on the free axis) so no on-device transposes are
needed anywhere; the host transposes inputs/outputs (cheap numpy).
"""

import numpy as np
import sys

sys.path.insert(0, "/opt/trn_rl_repo")

import concourse.bacc as bacc
import concourse.mybir as mybir
import concourse.tile as tile
from concourse import bass_utils

dt = mybir.dt
F = mybir.ActivationFunctionType
Alu = mybir.AluOpType

D = 1024       # d_model
S = 2048       # seq len (rows per batch)
Q = 512        # own query rows per core
H = 16         # heads
HD = 64        # head dim
INNER = 4096
P = 128
DC = D // P    # 8 d-chunks
IC = INNER // P  # 32 inner-chunks
EPS = 1e-5
NKH = 2        # key halves
KH = S // NKH  # keys per half (1024)

_BUILD_CACHE = {}


def _build(mm_r: bool):
    """Build the per-core Bass program. mm_r=True -> float32r matmuls."""
    mdt = dt.float32r if mm_r else dt.float32
    nc = bacc.Bacc("TRN2", target_bir_lowering=False, debug=False)

    # ---- DRAM I/O ----
    hT = nc.dram_tensor("hT", [D, S], dt.float32, kind="ExternalInput")
    hqT = nc.dram_tensor("hqT", [D, Q], dt.float32, kind="ExternalInput")
    maskT = nc.dram_tensor("maskT", [S, Q], dt.float32, kind="ExternalInput")
    w_qkv = nc.dram_tensor("w_qkv", [P, DC, 3 * D], dt.float32, kind="ExternalInput")
    w_proj = nc.dram_tensor("w_proj", [P, DC, D], dt.float32, kind="ExternalInput")
    w_fc = nc.dram_tensor("w_fc", [P, DC, INNER], dt.float32, kind="ExternalInput")
    w_mlp = nc.dram_tensor("w_mlp", [P, IC, D], dt.float32, kind="ExternalInput")
    # per-partition vectors, host-packed as [P, nchunks]
    bq = nc.dram_tensor("bq", [P, DC], dt.float32, kind="ExternalInput")
    bk = nc.dram_tensor("bk", [P, DC], dt.float32, kind="ExternalInput")
    bv = nc.dram_tensor("bv", [P, DC], dt.float32, kind="ExternalInput")
    bproj = nc.dram_tensor("bproj", [P, DC], dt.float32, kind="ExternalInput")
    bfc = nc.dram_tensor("bfc", [P, IC], dt.float32, kind="ExternalInput")
    bmlp = nc.dram_tensor("bmlp", [P, DC], dt.float32, kind="ExternalInput")
    g1 = nc.dram_tensor("g1", [P, DC], dt.float32, kind="ExternalInput")
    be1 = nc.dram_tensor("be1", [P, DC], dt.float32, kind="ExternalInput")
    g2 = nc.dram_tensor("g2", [P, DC], dt.float32, kind="ExternalInput")
    be2 = nc.dram_tensor("be2", [P, DC], dt.float32, kind="ExternalInput")
    outT = nc.dram_tensor("outT", [D, Q], dt.float32, kind="ExternalOutput")

    wdma = nc.gpsimd if mm_r else nc.sync  # casting DMA needed for fp32r

    with tile.TileContext(nc) as tc:
        with (
            tc.tile_pool(name="const", bufs=1) as const,
            tc.tile_pool(name="persist", bufs=1) as persist,
            tc.tile_pool(name="half", bufs=1) as half,
            tc.tile_pool(name="wstream", bufs=3) as wstream,
            tc.tile_pool(name="tmp", bufs=3) as tmp,
            tc.tile_pool(name="expp", bufs=3) as expp,
            tc.tile_pool(name="stats", bufs=2) as stats,
            tc.tile_pool(name="ps", bufs=3, space="PSUM") as ps,
            tc.tile_pool(name="ps_acc", bufs=2, space="PSUM") as ps_acc,
            tc.tile_pool(name="ps_bc", bufs=2, space="PSUM") as ps_bc,
        ):
            # ---- constants ----
            ones_col = const.tile([P, 1], dt.float32)     # lhsT for col-sums
            nc.vector.memset(ones_col[:], 1.0)
            ones_row = const.tile([1, P], dt.float32)     # lhsT for bcasts
            nc.vector.memset(ones_row[:], 1.0)

            def load_pvec(t):
                s = const.tile(list(t.shape), dt.float32, tag=t.name)
                nc.sync.dma_start(s[:], t[:])
                return s

            bq_s, bk_s, bv_s = load_pvec(bq), load_pvec(bk), load_pvec(bv)
            bproj_s, bfc_s, bmlp_s = load_pvec(bproj), load_pvec(bfc), load_pvec(bmlp)
            g1_s, be1_s = load_pvec(g1), load_pvec(be1)
            g2_s, be2_s = load_pvec(g2), load_pvec(be2)

            # ---------------------------------------------------------
            # layer-norm in transposed layout:
            #   x:[P, DC, N] fp32 -> out:[P, DC, N] (dtype odt)
            # reductions over the partition(+chunk) axis via ones-matmuls
            def layernorm_T(x_sb, N, g_s, be_s, odt, out_pool, tagp):
                nslices = N // 512
                mean = stats.tile([1, N], dt.float32, tag=f"mean{tagp}")
                var = stats.tile([1, N], dt.float32, tag=f"var{tagp}")
                for nsl in range(nslices):
                    nsli = slice(nsl * 512, (nsl + 1) * 512)
                    pss = ps.tile([1, 512], dt.float32, tag="lnsum")
                    psq = ps.tile([1, 512], dt.float32, tag="lnsq")
                    for c in range(DC):
                        sq = tmp.tile([P, 512], dt.float32, tag="sq")
                        nc.vector.tensor_tensor(
                            sq[:], x_sb[:, c, nsli], x_sb[:, c, nsli], Alu.mult
                        )
                        nc.tensor.matmul(
                            pss[:], ones_col[:], x_sb[:, c, nsli],
                            start=(c == 0), stop=(c == DC - 1),
                        )
                        nc.tensor.matmul(
                            psq[:], ones_col[:], sq[:],
                            start=(c == 0), stop=(c == DC - 1),
                        )
                    # mean = pss/D ; var = psq/D - mean^2
                    nc.vector.tensor_scalar_mul(mean[:, nsli], pss[:], 1.0 / D)
                    msq = stats.tile([1, 512], dt.float32, tag=f"msq{tagp}")
                    nc.vector.tensor_tensor(
                        msq[:], mean[:, nsli], mean[:, nsli], Alu.mult
                    )
                    nc.vector.scalar_tensor_tensor(
                        var[:, nsli], psq[:], 1.0 / D, msq[:], Alu.mult, Alu.subtract
                    )
                # rstd = 1/sqrt(var+eps)
                sd = stats.tile([1, N], dt.float32, tag=f"sd{tagp}")
                nc.scalar.activation(sd[:], var[:], F.Sqrt, bias=EPS)
                rstd = stats.tile([1, N], dt.float32, tag=f"rstd{tagp}")
                nc.vector.reciprocal(rstd[:], sd[:])
                # broadcast mean/rstd across partitions via PE outer product
                out = out_pool.tile([P, DC, N], odt, tag=f"lnout{tagp}")
                for nsl in range(nslices):
                    nsli = slice(nsl * 512, (nsl + 1) * 512)
                    mb = ps_bc.tile([P, 512], dt.float32, tag="mb")
                    rb = ps_bc.tile([P, 512], dt.float32, tag="rb")
                    nc.tensor.matmul(mb[:], ones_row[:], mean[:, nsli],
                                     start=True, stop=True)
                    nc.tensor.matmul(rb[:], ones_row[:], rstd[:, nsli],
                                     start=True, stop=True)
                    for c in range(DC):
                        t1 = tmp.tile([P, 512], dt.float32, tag="lnt1")
                        nc.vector.tensor_tensor(
                            t1[:], x_sb[:, c, nsli], mb[:], Alu.subtract
                        )
                        t2 = tmp.tile([P, 512], dt.float32, tag="lnt2")
                        nc.vector.scalar_tensor_tensor(
                            t2[:], t1[:], g_s[:, c : c + 1], rb[:], Alu.mult, Alu.mult
                        )
                        nc.vector.tensor_scalar_add(
                            out[:, c, nsli], t2[:], be_s[:, c : c + 1]
                        )
                return out

            # ---------------------------------------------------------
            # own-rows LN + QT
            hq_sb = persist.tile([P, DC, Q], dt.float32)
            nc.sync.dma_start(hq_sb[:], hqT.rearrange("(c p) n -> p c n", p=P))
            xq = layernorm_T(hq_sb, Q, g1_s, be1_s, mdt, persist, "q")
            qt = persist.tile([P, DC, Q], mdt)  # pair-packed head dims
            for p in range(DC):
                wk_t = wstream.tile([P, DC, P], mdt, tag="wq")
                wdma.dma_start(wk_t[:], w_qkv[:, :, p * P : (p + 1) * P])
                psq = ps.tile([P, Q], dt.float32, tag="qt")
                for c in range(DC):
                    nc.tensor.matmul(psq[:], wk_t[:, c, :], xq[:, c, :],
                                     start=(c == 0), stop=(c == DC - 1))
                nc.scalar.activation(qt[:, p, :], psq[:], F.Copy,
                                     bias=bq_s[:, p : p + 1])

            # attention accumulators (across key halves)
            attn_acc = persist.tile([P, DC, Q], dt.float32)
            sums_acc = persist.tile([1, H, Q], dt.float32)

            for kh in range(NKH):
                khl = slice(kh * KH, (kh + 1) * KH)
                hh = half.tile([P, DC, KH], dt.float32, tag="hh")
                nc.sync.dma_start(
                    hh[:], hT.rearrange("(c p) n -> p c n", p=P)[:, :, khl]
                )
                xln = layernorm_T(hh, KH, g1_s, be1_s, mdt, half, "h")

                # KT for this half: [P(pair dims), DC(pair), KH(keys)]
                kt_sb = half.tile([P, DC, KH], mdt, tag="kt")
                for p in range(DC):
                    wk_t = wstream.tile([P, DC, P], mdt, tag="wk")
                    wdma.dma_start(wk_t[:], w_qkv[:, :, D + p * P : D + (p + 1) * P])
                    for nsl in range(KH // 512):
                        nsli = slice(nsl * 512, (nsl + 1) * 512)
                        psk = ps.tile([P, 512], dt.float32, tag="kt")
                        for c in range(DC):
                            nc.tensor.matmul(psk[:], wk_t[:, c, :], xln[:, c, nsli],
                                             start=(c == 0), stop=(c == DC - 1))
                        nc.scalar.activation(kt_sb[:, p, nsli], psk[:], F.Copy,
                                             bias=bk_s[:, p : p + 1])

                # V for this half: [P(keys), KH//P(ktile), H*65] (65th col = ones)
                NKT = KH // P  # 8 key tiles per half
                v_sb = half.tile([P, NKT, H * 65], mdt, tag="v")
                for vs in range(2):
                    vsl = slice(vs * 512, (vs + 1) * 512)
                    wv_t = wstream.tile([P, DC, 512], mdt, tag="wv")
                    wdma.dma_start(wv_t[:], w_qkv[:, :, 2 * D + vs * 512 : 2 * D + (vs + 1) * 512])
                    for kt in range(NKT):
                        psv = ps.tile([P, 512], dt.float32, tag="v")
                        for c in range(DC):
                            nc.tensor.matmul(
                                psv[:], xln[:, c, kt * P : (kt + 1) * P],
                                wv_t[:, c, :],
                                start=(c == 0), stop=(c == DC - 1),
                            )
                        dst = v_sb[:, kt, vs * 8 * 65 : (vs + 1) * 8 * 65].rearrange(
                            "p (h x) -> p h x", x=65
                        )[:, :, 0:64]
                        nc.scalar.activation(
                            dst, psv[:].rearrange("p (h x) -> p h x", x=64),
                            F.Copy,
                            bias=bv_s[:, vs * 4 : vs * 4 + 4].rearrange(
                                "p c -> p c 1"
                            ).to_broadcast([P, 4, 2, 64]).rearrange(
                                "p c t x -> p (c t) x"
                            ),
                        )
                    # note: bias here adds b_v; but attn = probs@(xWv) + b_v
                    # needs b_v added *after* softmax-weighted sum, and since
                    # sum(probs)=1 adding to V pre-sum is equivalent ONLY for
                    # unnormalized... we fold b_v at eviction instead; bias=0.
                # ones column for denominator
                vview = v_sb[:].rearrange("p k (h x) -> p k h x", x=65)
                nc.vector.memset(vview[:, :, :, 64:65], 1.0)

                # mask for this half
                mask_sb = half.tile([P, NKT, Q], dt.float32, tag="mask")
                nc.sync.dma_start(
                    mask_sb[:], maskT.rearrange("(k p) n -> p k n", p=P)[:, kh * NKT : (kh + 1) * NKT, :]
                )

                for h in range(H):
                    hp, hs = h // 2, (h % 2) * 64
                    pa = ps_acc.tile([65, Q], dt.float32, tag="pv")
                    for kt in range(NKT):
                        pss = ps.tile([P, Q], dt.float32, tag="sc")
                        nc.tensor.matmul(
                            pss[:],
                            kt_sb[hs : hs + 64, hp, kt * P : (kt + 1) * P],
                            qt[hs : hs + 64, hp, :],
                            start=True, stop=True,
                        )
                        nc.vector.tensor_tensor(
                            pss[:], pss[:], mask_sb[:, kt, :], Alu.add
                        )
                        et = expp.tile([P, Q], mdt, tag="exp")
                        nc.scalar.activation(et[:], pss[:], F.Exp, scale=0.125)
                        nc.tensor.matmul(
                            pa[:], v_sb[:, kt, h * 65 : h * 65 + 65], et[:],
                            start=(kt == 0), stop=(kt == NKT - 1),
                        )
                    if kh == 0:
                        nc.scalar.activation(
                            attn_acc[hs : hs + 64, hp, :], pa[0:64, :], F.Copy
                        )
                        nc.scalar.activation(
                            sums_acc[:, h, :], pa[64:65, :], F.Copy
                        )
                    else:
                        nc.vector.tensor_tensor(
                            attn_acc[hs : hs + 64, hp, :],
                            attn_acc[hs : hs + 64, hp, :], pa[0:64, :], Alu.add,
                        )
                        nc.vector.tensor_tensor(
                            sums_acc[:, h, :], sums_acc[:, h, :], pa[64:65, :],
                            Alu.add,
                        )

            # normalize + v-bias -> attnT (pair-packed [P, DC, Q])
            attnT = persist.tile([P, DC, Q], mdt)
            recip = stats.tile([1, H, Q], dt.float32, tag="recip")
            nc.vector.reciprocal(recip[:], sums_acc[:])
            for h in range(H):
                hp, hs = h // 2, (h % 2) * 64
                bc = ps_bc.tile([64, Q], dt.float32, tag="rbc")
                nc.tensor.matmul(bc[:], ones_row[:, 0:64], recip[:, h, :],
                                 start=True, stop=True)
                t1 = tmp.tile([64, Q], dt.float32, tag="anorm")
                nc.vector.tensor_tensor(
                    t1[:], attn_acc[hs : hs + 64, hp, :], bc[:], Alu.mult
                )
                nc.vector.tensor_scalar_add(
                    attnT[hs : hs + 64, hp, :], t1[:],
                    bv_s[hs : hs + 64, hp : hp + 1],
                )

            # ---- proj + residual -> h2 (fp32) ----
            wproj_s = persist.tile([P, DC, D], mdt, tag="wproj")
            wdma.dma_start(wproj_s[:], w_proj[:])
            h2 = persist.tile([P, DC, Q], dt.float32, tag="h2")
            for mo in range(DC):
                psp = ps.tile([P, Q], dt.float32, tag="proj")
                for c in range(DC):
                    nc.tensor.matmul(
                        psp[:], wproj_s[:, c, mo * P : (mo + 1) * P],
                        attnT[:, c, :],
                        start=(c == 0), stop=(c == DC - 1),
                    )
                nc.vector.scalar_tensor_tensor(
                    h2[:, mo, :], psp[:], bproj_s[:, mo : mo + 1],
                    hq_sb[:, mo, :], Alu.add, Alu.add,
                )

            # ---- LN2 -> h2n ----
            h2n = layernorm_T(h2, Q, g2_s, be2_s, mdt, persist, "2")

            # ---- fc + gelu -> g  [P, IC, Q] ----
            g_sb = persist.tile([P, IC, Q], mdt, tag="g")
            for m in range(IC):
                wfc_t = wstream.tile([P, DC, P], mdt, tag="wfc")
                wdma.dma_start(wfc_t[:], w_fc[:, :, m * P : (m + 1) * P])
                psf = ps.tile([P, Q], dt.float32, tag="fc")
                for c in range(DC):
                    nc.tensor.matmul(psf[:], wfc_t[:, c, :], h2n[:, c, :],
                                     start=(c == 0), stop=(c == DC - 1))
                nc.scalar.activation(g_sb[:, m, :], psf[:], F.Gelu,
                                     bias=bfc_s[:, m : m + 1])

            # ---- mlp + residual -> out ----
            out_sb = persist.tile([P, DC, Q], dt.float32, tag="out")
            for mo in range(DC):
                wm_t = wstream.tile([P, IC, P], mdt, tag="wmlp")
                wdma.dma_start(wm_t[:], w_mlp[:, :, mo * P : (mo + 1) * P])
                psm = ps.tile([P, Q], dt.float32, tag="mlp")
                for c in range(IC):
                    nc.tensor.matmul(psm[:], wm_t[:, c, :], g_sb[:, c, :],
                                     start=(c == 0), stop=(c == IC - 1))
                nc.vector.scalar_tensor_tensor(
                    out_sb[:, mo, :], psm[:], bmlp_s[:, mo : mo + 1],
                    h2[:, mo, :], Alu.add, Alu.add,
                )
                nc.sync.dma_start(
                    outT.rearrange("(c p) n -> p c n", p=P)[:, mo, :],
                    out_sb[:, mo, :],
                )

    nc.compile()
    return nc


def _get_nc(mm_r: bool):
    if mm_r not in _BUILD_CACHE:
        _BUILD_CACHE[mm_r] = _build(mm_r)
    return _BUILD_CACHE[mm_r]


def _prep_in_maps(inputs):
    h = np.asarray(inputs["hidden_states"], dtype=np.float32)  # [2, 2048, 1024]
    w_qkv = np.asarray(inputs["w_qkv"], np.float32)
    b_qkv = np.asarray(inputs["b_qkv"], np.float32)

    def chunk_w(w):  # [D_in, N] -> [P, D_in//P, N]
        return np.ascontiguousarray(
            w.reshape(-1, P, w.shape[1]).transpose(1, 0, 2)
        )

    def pvec(v):  # [n*P] -> [P, n]
        return np.ascontiguousarray(v.reshape(-1, P).T)

    shared = {
        "w_qkv": chunk_w(w_qkv),
        "w_proj": chunk_w(np.asarray(inputs["w_proj"], np.float32)),
        "w_fc": chunk_w(np.asarray(inputs["w_fc"], np.float32)),
        "w_mlp": chunk_w(np.asarray(inputs["w_mlp"], np.float32)),
        "bq": pvec(b_qkv[0:D]),
        "bk": pvec(b_qkv[D : 2 * D]),
        "bv": pvec(b_qkv[2 * D : 3 * D]),
        "bproj": pvec(np.asarray(inputs["b_proj"], np.float32)),
        "bfc": pvec(np.asarray(inputs["b_fc"], np.float32)),
        "bmlp": pvec(np.asarray(inputs["b_mlp"], np.float32)),
        "g1": pvec(np.asarray(inputs["g1"], np.float32)),
        "be1": pvec(np.asarray(inputs["be1"], np.float32)),
        "g2": pvec(np.asarray(inputs["g2"], np.float32)),
        "be2": pvec(np.asarray(inputs["be2"], np.float32)),
    }
    in_maps = []
    for core in range(8):
        b, j = core // 4, core % 4
        rows = slice(j * Q, (j + 1) * Q)
        km = np.arange(S)[:, None] <= (j * Q + np.arange(Q))[None, :]
        maskT = np.where(km, np.float32(0.0), np.float32(-10000.0))
        in_maps.append(
            dict(
                shared,
                hT=np.ascontiguousarray(h[b].T),
                hqT=np.ascontiguousarray(h[b, rows].T),
                maskT=np.ascontiguousarray(maskT.astype(np.float32)),
            )
        )
    return in_maps


def _stitch(results):
    out = np.empty((2, S, D), dtype=np.float32)
    for core in range(8):
        b, j = core // 4, core % 4
        out[b, j * Q : (j + 1) * Q] = results[core]["outT"].T
    return out


def run(inputs, mm_r=True, trace=False, trace_cores=None):
    nc = _get_nc(mm_r)
    in_maps = _prep_in_maps(inputs)
    res = bass_utils.run_bass_kernel_spmd(
        nc, in_maps, core_ids=list(range(8)), trace=trace,
        trace_cores=trace_cores,
    )
    return _stitch(res.results), res


def kernel(**inputs) -> np.ndarray:
    out, _ = run(inputs, mm_r=True)
    return out


# revision 20
# speedup vs baseline: 1.2925x; 1.2925x over previous
"""GPT2 block kernel for 8 TRN2 NeuronCores (Bass/Tile, SPMD).

Sharding: the 4096 rows (batch*seq) are split 8 ways -> 512 rows/core
(4 cores per batch element). Each core redundantly computes K,V for its
batch, then causal attention for its own 512 query rows against all
2048 keys, then proj/LN2/MLP for its own rows only. Zero collectives.

All tensors are kept in a "transposed" layout (feature dim on SBUF
partitions, token dim on the free axis) so no on-device transposes are
needed. The host transposes inputs/outputs and rotates each core's key
order so its own rows are always key-quarter 0 (attention is
permutation-invariant under a matching mask, which is passed as data).
"""

import numpy as np
import sys

sys.path.insert(0, "/opt/trn_rl_repo")

import concourse.bacc as bacc
import concourse.mybir as mybir
import concourse.tile as tile
from concourse import bass_utils

dt = mybir.dt
F = mybir.ActivationFunctionType
Alu = mybir.AluOpType

D = 1024
S = 2048
Q = 512        # own rows per core
H = 16
HD = 64
INNER = 4096
P = 128
DC = D // P    # 8
IC = INNER // P  # 32
EPS = 1e-5
NQT = 4        # key quarters
KQ = S // NQT  # 512 keys per quarter
NKT = KQ // P  # 4 key tiles per quarter

_BUILD_CACHE = {}


def _build(mm_r: bool):
    mdt = dt.float32r if mm_r else dt.float32
    nc = bacc.Bacc("TRN2", target_bir_lowering=False, debug=False)

    hT = nc.dram_tensor("hT", [D, S], dt.float32, kind="ExternalInput")
    maskband = nc.dram_tensor("maskband", [Q, Q], dt.float32, kind="ExternalInput")
    maskb = nc.dram_tensor("maskb", [P, NQT * NKT], dt.float32, kind="ExternalInput")
    # weights arrive pre-tiled from the host in the exact consumption
    # order so every weight DMA is fully contiguous on HWDGE
    w_q = nc.dram_tensor("w_q", [DC, P, DC, P], mdt, kind="ExternalInput")
    w_k = nc.dram_tensor("w_k", [DC, P, DC, P], mdt, kind="ExternalInput")
    w_v = nc.dram_tensor("w_v", [4, P, DC, 256], mdt, kind="ExternalInput")
    w_projr = nc.dram_tensor("w_projr", [DC, HD, H, P], mdt, kind="ExternalInput")
    w_fcr = nc.dram_tensor("w_fcr", [IC, P, DC, P], mdt, kind="ExternalInput")
    w_mlpr = nc.dram_tensor("w_mlpr", [2, DC, P, IC // 2, P], mdt, kind="ExternalInput")
    bq = nc.dram_tensor("bq", [P, DC], dt.float32, kind="ExternalInput")
    bk = nc.dram_tensor("bk", [P, DC], dt.float32, kind="ExternalInput")
    bv = nc.dram_tensor("bv", [HD, H], dt.float32, kind="ExternalInput")
    bproj = nc.dram_tensor("bproj", [P, DC], dt.float32, kind="ExternalInput")
    bfc = nc.dram_tensor("bfc", [P, IC], dt.float32, kind="ExternalInput")
    bmlp = nc.dram_tensor("bmlp", [P, DC], dt.float32, kind="ExternalInput")
    g1 = nc.dram_tensor("g1", [P, DC], dt.float32, kind="ExternalInput")
    be1 = nc.dram_tensor("be1", [P, DC], dt.float32, kind="ExternalInput")
    g2 = nc.dram_tensor("g2", [P, DC], dt.float32, kind="ExternalInput")
    be2 = nc.dram_tensor("be2", [P, DC], dt.float32, kind="ExternalInput")
    outT = nc.dram_tensor("outT", [D, Q], dt.float32, kind="ExternalOutput")

    hT_r = hT.rearrange("(c p) n -> p c n", p=P)
    maskband_r = maskband.rearrange("(k p) n -> p k n", p=P)

    with tile.TileContext(nc) as tc:
        with (
            tc.tile_pool(name="const", bufs=1) as const,
            tc.tile_pool(name="stats", bufs=1) as stats,
            tc.tile_pool(name="tmp", bufs=2) as tmp,
            tc.tile_pool(name="hstream", bufs=3) as hstream,
            tc.tile_pool(name="persist", bufs=1) as persist,
            tc.tile_pool(name="ps", bufs=3, space="PSUM") as ps,
            tc.tile_pool(name="lnps", bufs=2, space="PSUM") as lnps,
        ):
            ones_col = const.tile([P, 1], dt.float32)
            nc.vector.memset(ones_col[:], 1.0)
            ones_row = const.tile([1, P], dt.float32)
            nc.vector.memset(ones_row[:], 1.0)
            ones65 = const.tile([65, HD], dt.float32)
            nc.vector.memset(ones65[:], 1.0)
            eps_t = const.tile([1, 1], dt.float32)
            nc.vector.memset(eps_t[:], EPS)

            def load_pvec(t):
                s = const.tile(list(t.shape), dt.float32, tag=t.name)
                nc.sync.dma_start(s[:], t[:])
                return s

            maskb_s = load_pvec(maskb)
            bq_s, bk_s, bv_s = load_pvec(bq), load_pvec(bk), load_pvec(bv)
            bproj_s, bfc_s, bmlp_s = load_pvec(bproj), load_pvec(bfc), load_pvec(bmlp)
            g1_s, be1_s = load_pvec(g1), load_pvec(be1)
            g2_s, be2_s = load_pvec(g2), load_pvec(be2)

            # LN in transposed layout. get_chunk(c, keep) returns a [P, Q]
            # fp32 AP for chunk c (called for stats pass and apply pass).
            # Column stats via ones-matmuls; mean/rstd broadcast across
            # partitions via PE outer products.
            def layernorm_T(get_chunk, g_s, be_s, odt, out_pool, tag):
                pss = lnps.tile([1, Q], dt.float32, tag="lnps")
                psq = lnps.tile([1, Q], dt.float32, tag="lnps")
                acc = tmp.tile([P, Q], dt.float32, tag="lnacc")
                accq = tmp.tile([P, Q], dt.float32, tag="lnaccq")
                for c in range(DC):
                    xc = get_chunk(c)
                    if c == 0:
                        nc.vector.tensor_copy(acc[:], xc)
                        nc.vector.tensor_tensor(accq[:], xc, xc, Alu.mult)
                    else:
                        nc.vector.tensor_tensor(acc[:], acc[:], xc, Alu.add)
                        sq = tmp.tile([P, Q], dt.float32, tag="sq")
                        nc.vector.tensor_tensor(sq[:], xc, xc, Alu.mult)
                        nc.vector.tensor_tensor(accq[:], accq[:], sq[:], Alu.add)
                nc.tensor.matmul(pss[:], ones_col[:], acc[:], start=True, stop=True)
                nc.tensor.matmul(psq[:], ones_col[:], accq[:], start=True, stop=True)
                mean = stats.tile([1, Q], dt.float32, tag="mean")
                nc.vector.tensor_scalar_mul(mean[:], pss[:], 1.0 / D)
                msq = stats.tile([1, Q], dt.float32, tag="msq")
                nc.vector.tensor_tensor(msq[:], mean[:], mean[:], Alu.mult)
                var = stats.tile([1, Q], dt.float32, tag="var")
                nc.vector.scalar_tensor_tensor(
                    var[:], psq[:], 1.0 / D, msq[:], Alu.mult, Alu.subtract
                )
                nc.scalar.activation(msq[:], var[:], F.Sqrt, bias=eps_t[:])
                nc.vector.reciprocal(msq[:], msq[:])  # msq now holds rstd
                mb = lnps.tile([P, Q], dt.float32, tag="lnps")
                rb = lnps.tile([P, Q], dt.float32, tag="lnps")
                nc.tensor.matmul(mb[:], ones_row[:], mean[:], start=True, stop=True)
                nc.tensor.matmul(rb[:], ones_row[:], msq[:], start=True, stop=True)
                out = out_pool.tile([P, DC, Q], odt, tag=tag)
                for c in range(DC):
                    xc = get_chunk(c)
                    t1 = tmp.tile([P, Q], dt.float32, tag="lnt1")
                    nc.vector.tensor_tensor(t1[:], xc, mb[:], Alu.subtract)
                    nc.vector.scalar_tensor_tensor(
                        out[:, c, :], t1[:], g_s[:, c : c + 1], rb[:],
                        Alu.mult, Alu.mult,
                    )
                    nc.vector.tensor_scalar_add(
                        out[:, c, :], out[:, c, :], be_s[:, c : c + 1]
                    )
                return out

            def resident_chunks(x_sb):
                return lambda c: x_sb[:, c, :]

            h2 = persist.tile([P, DC, Q], dt.float32, tag="h2")

            with tc.tile_pool(name="attnsc", bufs=1) as attnsc:
                qt = attnsc.tile([P, DC, Q], mdt, tag="qt")
                attn_acc = attnsc.tile([65, H, Q], dt.float32, tag="attn_acc")
                v_sb = attnsc.tile([P, NKT, H * 65], mdt, tag="v")
                vview = v_sb[:].rearrange("p k (h x) -> p k h x", x=65)
                nc.vector.tensor_copy(
                    vview[:, :, :, 64:65],
                    ones_col[:].to_broadcast([P, NKT, H, 1]),
                )
                hq_sb = attnsc.tile([P, DC, Q], dt.float32, tag="hq")
                for c in range(DC):
                    nc.sync.dma_start(hq_sb[:, c, :], hT_r[:, c, 0:Q])

                with (
                    tc.tile_pool(name="quarter", bufs=1) as quarter,
                    tc.tile_pool(name="wkv", bufs=2) as wkv,
                    tc.tile_pool(name="expp", bufs=3) as expp,
                    tc.tile_pool(name="pvps", bufs=2, space="PSUM") as pvps,
                ):
                    for q in range(NQT):
                        qsl = slice(q * KQ, (q + 1) * KQ)
                        if q == 0:
                            get_chunk = resident_chunks(hq_sb)
                        else:
                            def get_chunk(c, qsl=qsl):
                                hc = hstream.tile([P, Q], dt.float32, tag="hhc")
                                nc.sync.dma_start(hc[:], hT_r[:, c, qsl])
                                return hc[:]
                        xln = layernorm_T(get_chunk, g1_s, be1_s, mdt,
                                          quarter, "xln")

                        if q == 0:
                            for p in range(DC):
                                wq_t = wkv.tile([P, DC, P], mdt, tag="wq")
                                nc.sync.dma_start(wq_t[:], w_q[p])
                                psq_ = ps.tile([P, Q], dt.float32, tag="mm")
                                for c in range(DC):
                                    nc.tensor.matmul(
                                        psq_[:], wq_t[:, c, :], xln[:, c, :],
                                        start=(c == 0), stop=(c == DC - 1),
                                    )
                                nc.scalar.activation(
                                    qt[:, p, :], psq_[:], F.Identity,
                                    bias=bq_s[:, p : p + 1],
                                )

                        kt_sb = quarter.tile([P, DC, KQ], mdt, tag="kt")
                        for p in range(DC):
                            wk_t = wkv.tile([P, DC, P], mdt, tag="wq")
                            nc.sync.dma_start(wk_t[:], w_k[p])
                            psk = ps.tile([P, Q], dt.float32, tag="mm")
                            for c in range(DC):
                                nc.tensor.matmul(
                                    psk[:], wk_t[:, c, :], xln[:, c, :],
                                    start=(c == 0), stop=(c == DC - 1),
                                )
                            nc.scalar.activation(
                                kt_sb[:, p, :], psk[:], F.Identity,
                                bias=bk_s[:, p : p + 1],
                            )

                        for vs in range(4):
                            wv_t = wkv.tile([P, DC, 256], mdt, tag="wv")
                            nc.sync.dma_start(wv_t[:], w_v[vs])
                            for kt in range(NKT):
                                psv = ps.tile([P, Q], dt.float32, tag="mm")
                                for c in range(DC):
                                    nc.tensor.matmul(
                                        psv[:, 0:256],
                                        xln[:, c, kt * P : (kt + 1) * P],
                                        wv_t[:, c, :],
                                        start=(c == 0), stop=(c == DC - 1),
                                    )
                                dst = v_sb[
                                    :, kt, vs * 4 * 65 : (vs + 1) * 4 * 65
                                ].rearrange("p (h x) -> p h x", x=65)[:, :, 0:64]
                                nc.scalar.activation(
                                    dst,
                                    psv[:, 0:256].rearrange("p (h x) -> p h x", x=64),
                                    F.Copy,
                                )

                        if q == 0:
                            mask_q = quarter.tile([P, NKT, Q], dt.float32,
                                                  tag="mask")
                            nc.sync.dma_start(mask_q[:], maskband_r[:])

                        for h in range(H):
                            hp, hs = h // 2, (h % 2) * 64
                            pa = pvps.tile([65, Q], dt.float32, tag="pv")
                            for kt in range(NKT):
                                pss = ps.tile([P, Q], dt.float32, tag="mm")
                                nc.tensor.matmul(
                                    pss[:],
                                    kt_sb[hs : hs + 64, hp, kt * P : (kt + 1) * P],
                                    qt[hs : hs + 64, hp, :],
                                    start=True, stop=True,
                                )
                                if q == 0:
                                    nc.vector.tensor_tensor(
                                        pss[:], pss[:], mask_q[:, kt, :], Alu.add
                                    )
                                et = expp.tile([P, Q], mdt, tag="exp")
                                nc.scalar.activation(
                                    et[:], pss[:], F.Exp, scale=0.125,
                                    bias=maskb_s[:, q * NKT + kt : q * NKT + kt + 1],
                                )
                                nc.tensor.matmul(
                                    pa[:], v_sb[:, kt, h * 65 : h * 65 + 65],
                                    et[:],
                                    start=(kt == 0), stop=(kt == NKT - 1),
                                )
                            if q == 0:
                                nc.scalar.activation(
                                    attn_acc[:, h, :], pa[:], F.Copy
                                )
                            else:
                                nc.vector.tensor_tensor(
                                    attn_acc[:, h, :], attn_acc[:, h, :],
                                    pa[:], Alu.add,
                                )

                # normalize per head -> attnT [64, H, Q], then proj as a
                # plain GEMM over the 16 head-chunks + residual -> h2.
                with tc.tile_pool(name="projsc", bufs=1) as projsc, \
                     tc.tile_pool(name="pstream", bufs=2) as pstream:
                    attnT = projsc.tile([HD, H, Q], mdt, tag="attnT")
                    for h in range(H):
                        nc.vector.reciprocal(
                            attn_acc[64:65, h, :], attn_acc[64:65, h, :]
                        )
                        bc = lnps.tile([P, Q], dt.float32, tag="lnps")
                        nc.tensor.matmul(
                            bc[0:64, :], ones65[64:65, :],
                            attn_acc[64:65, h, :], start=True, stop=True,
                        )
                        t1 = tmp.tile([HD, Q], dt.float32, tag="anorm")
                        nc.vector.tensor_tensor(
                            t1[:], attn_acc[0:64, h, :], bc[0:64, :], Alu.mult
                        )
                        nc.vector.tensor_scalar_add(
                            attnT[:, h, :], t1[:], bv_s[:, h : h + 1]
                        )
                    for mo in range(DC):
                        wp_t = pstream.tile([HD, H, P], mdt, tag="wp")
                        nc.sync.dma_start(wp_t[:], w_projr[mo])
                        psp = ps.tile([P, Q], dt.float32, tag="mm")
                        for c in range(H):
                            nc.tensor.matmul(
                                psp[:], wp_t[:, c, :], attnT[:, c, :],
                                start=(c == 0), stop=(c == H - 1),
                            )
                        nc.vector.scalar_tensor_tensor(
                            h2[:, mo, :], psp[:], bproj_s[:, mo : mo + 1],
                            hq_sb[:, mo, :], Alu.add, Alu.add,
                        )

            # ---- LN2 / fc+gelu / mlp + residual ----
            with (
                tc.tile_pool(name="mlpsc", bufs=1) as mlpsc,
                tc.tile_pool(name="wfcs", bufs=3) as wfcs,
                tc.tile_pool(name="wmlps", bufs=2) as wmlps,
            ):
                h2n = layernorm_T(resident_chunks(h2), g2_s, be2_s, mdt,
                                  mlpsc, "h2n")
                y2 = mlpsc.tile([P, DC, Q], dt.float32, tag="y2")
                g_half = mlpsc.tile([P, IC // 2, Q], mdt, tag="g")
                for ih in range(2):
                    for m in range(IC // 2):
                        mg = ih * (IC // 2) + m
                        wfc_t = wfcs.tile([P, DC, P], mdt, tag="wfc")
                        nc.sync.dma_start(wfc_t[:], w_fcr[mg])
                        psf = ps.tile([P, Q], dt.float32, tag="mm")
                        for c in range(DC):
                            nc.tensor.matmul(
                                psf[:], wfc_t[:, c, :], h2n[:, c, :],
                                start=(c == 0), stop=(c == DC - 1),
                            )
                        nc.scalar.activation(
                            g_half[:, m, :], psf[:], F.Gelu,
                            bias=bfc_s[:, mg : mg + 1],
                        )
                    for mo in range(DC):
                        wm_t = wmlps.tile([P, IC // 2, P], mdt, tag="wmlp")
                        nc.sync.dma_start(wm_t[:], w_mlpr[ih, mo])
                        psm = ps.tile([P, Q], dt.float32, tag="mm")
                        for c in range(IC // 2):
                            nc.tensor.matmul(
                                psm[:], wm_t[:, c, :], g_half[:, c, :],
                                start=(c == 0), stop=(c == IC // 2 - 1),
                            )
                        if ih == 0:
                            nc.scalar.activation(y2[:, mo, :], psm[:], F.Copy)
                        else:
                            ot = tmp.tile([P, Q], dt.float32, tag="outt")
                            nc.vector.tensor_tensor(
                                ot[:], y2[:, mo, :], psm[:], Alu.add
                            )
                            nc.vector.scalar_tensor_tensor(
                                ot[:], ot[:], bmlp_s[:, mo : mo + 1],
                                h2[:, mo, :], Alu.add, Alu.add,
                            )
                            nc.sync.dma_start(
                                outT.rearrange("(c p) n -> p c n", p=P)[:, mo, :],
                                ot[:],
                            )

    nc.compile()
    return nc


def _get_nc(mm_r: bool):
    if mm_r not in _BUILD_CACHE:
        _BUILD_CACHE[mm_r] = _build(mm_r)
    return _BUILD_CACHE[mm_r]


def _prep_in_maps(inputs):
    h = np.asarray(inputs["hidden_states"], dtype=np.float32)
    b_qkv = np.asarray(inputs["b_qkv"], np.float32)
    w_qkv = np.asarray(inputs["w_qkv"], np.float32)

    def chunk_w(w, p=P):  # [Din, N] -> [p, Din//p, N]
        return np.ascontiguousarray(w.reshape(-1, p, w.shape[1]).transpose(1, 0, 2))

    def pvec(v, p=P):  # [n*p] -> [p, n]
        return np.ascontiguousarray(v.reshape(-1, p).T)

    def mslice(a, nsl):  # [p, c, n] -> [n//nsl, p, c, nsl]
        p, c, n = a.shape
        return np.ascontiguousarray(
            a.reshape(p, c, n // nsl, nsl).transpose(2, 0, 1, 3)
        )

    wq = mslice(chunk_w(w_qkv[:, 0:D]), P)
    wk = mslice(chunk_w(w_qkv[:, D : 2 * D]), P)
    wv = mslice(chunk_w(w_qkv[:, 2 * D : 3 * D]), 256)
    w_proj = np.asarray(inputs["w_proj"], np.float32)
    wp = np.ascontiguousarray(
        w_proj.reshape(H, HD, DC, P).transpose(2, 1, 0, 3)
    )
    wfc = mslice(chunk_w(np.asarray(inputs["w_fc"], np.float32)), P)
    wm = chunk_w(np.asarray(inputs["w_mlp"], np.float32))  # [128, 32, 1024]
    wmlp = np.ascontiguousarray(
        wm.reshape(P, 2, IC // 2, DC, P).transpose(1, 3, 0, 2, 4)
    )
    vis = np.arange(Q)[:, None] <= np.arange(Q)[None, :]  # key i visible to query u
    maskband = np.where(vis, np.float32(0.0), np.float32(-10000.0))

    shared = {
        "w_q": wq, "w_k": wk, "w_v": wv, "w_projr": wp,
        "w_fcr": wfc, "w_mlpr": wmlp,
        "maskband": np.ascontiguousarray(maskband.astype(np.float32)),
        "bq": pvec(b_qkv[0:D]),
        "bk": pvec(b_qkv[D : 2 * D]),
        "bv": pvec(b_qkv[2 * D : 3 * D], p=HD),
        "bproj": pvec(np.asarray(inputs["b_proj"], np.float32)),
        "bfc": pvec(np.asarray(inputs["b_fc"], np.float32)),
        "bmlp": pvec(np.asarray(inputs["b_mlp"], np.float32)),
        "g1": pvec(np.asarray(inputs["g1"], np.float32)),
        "be1": pvec(np.asarray(inputs["be1"], np.float32)),
        "g2": pvec(np.asarray(inputs["g2"], np.float32)),
        "be2": pvec(np.asarray(inputs["be2"], np.float32)),
    }
    in_maps = []
    for core in range(8):
        b, j = core // 4, core % 4
        perm = (np.arange(S) + j * Q) % S  # own rows become keys 0..511
        hrot = h[b, perm]
        # per-key exp bias: -10000/8 for keys strictly after the own
        # block (never visible); 0 otherwise (quarter 0 is handled by
        # the triangular band mask).
        masked = perm >= (j + 1) * Q
        mb = np.where(masked, np.float32(-1250.0), np.float32(0.0))
        maskb = np.ascontiguousarray(mb.reshape(NQT * NKT, P).T)
        in_maps.append(
            dict(
                shared,
                hT=np.ascontiguousarray(hrot.T),
                maskb=maskb.astype(np.float32),
            )
        )
    return in_maps


def _stitch(results):
    out = np.empty((2, S, D), dtype=np.float32)
    for core in range(8):
        b, j = core // 4, core % 4
        out[b, j * Q : (j + 1) * Q] = results[core]["outT"].T
    return out


def run(inputs, mm_r=True, trace=False, trace_cores=None):
    nc = _get_nc(mm_r)
    in_maps = _prep_in_maps(inputs)
    res = bass_utils.run_bass_kernel_spmd(
        nc, in_maps, core_ids=list(range(8)), trace=trace, trace_cores=trace_cores
    )
    return _stitch(res.results), res


def kernel(**inputs) -> np.ndarray:
    out, _ = run(inputs, mm_r=True)
    return out


# revision 22
# speedup vs baseline: 1.3133x; 1.0161x over previous
"""GPT2 block kernel for 8 TRN2 NeuronCores (Bass/Tile, SPMD).

Sharding: the 4096 rows (batch*seq) are split 8 ways -> 512 rows/core
(4 cores per batch element). Each core redundantly computes K,V for its
batch, then causal attention for its own 512 query rows against all
2048 keys, then proj/LN2/MLP for its own rows only. Zero collectives.

All tensors are kept in a "transposed" layout (feature dim on SBUF
partitions, token dim on the free axis) so no on-device transposes are
needed. The host transposes inputs/outputs and rotates each core's key
order so its own rows are always key-quarter 0 (attention is
permutation-invariant under a matching mask, which is passed as data).
"""

import numpy as np
import sys

sys.path.insert(0, "/opt/trn_rl_repo")

import concourse.bacc as bacc
import concourse.mybir as mybir
import concourse.tile as tile
from concourse import bass_utils

dt = mybir.dt
F = mybir.ActivationFunctionType
Alu = mybir.AluOpType

D = 1024
S = 2048
Q = 512        # own rows per core
H = 16
HD = 64
INNER = 4096
P = 128
DC = D // P    # 8
IC = INNER // P  # 32
EPS = 1e-5
NQT = 4        # key quarters
KQ = S // NQT  # 512 keys per quarter
NKT = KQ // P  # 4 key tiles per quarter

_BUILD_CACHE = {}


def _build(mm_r: bool):
    mdt = dt.float32r if mm_r else dt.float32
    nc = bacc.Bacc("TRN2", target_bir_lowering=False, debug=False)

    hT = nc.dram_tensor("hT", [D, S], dt.float32, kind="ExternalInput")
    maskband = nc.dram_tensor("maskband", [Q, Q], dt.float32, kind="ExternalInput")
    maskb = nc.dram_tensor("maskb", [P, NQT * NKT], dt.float32, kind="ExternalInput")
    # weights arrive pre-tiled from the host in the exact consumption
    # order so every weight DMA is fully contiguous on HWDGE
    w_q = nc.dram_tensor("w_q", [DC, P, DC, P], mdt, kind="ExternalInput")
    w_k = nc.dram_tensor("w_k", [DC, P, DC, P], mdt, kind="ExternalInput")
    w_v = nc.dram_tensor("w_v", [2, P, DC, 512], mdt, kind="ExternalInput")
    w_projr = nc.dram_tensor("w_projr", [DC, HD, H, P], mdt, kind="ExternalInput")
    w_fcr = nc.dram_tensor("w_fcr", [IC, P, DC, P], mdt, kind="ExternalInput")
    w_mlpr = nc.dram_tensor("w_mlpr", [2, DC, P, IC // 2, P], mdt, kind="ExternalInput")
    bq = nc.dram_tensor("bq", [P, DC], dt.float32, kind="ExternalInput")
    bk = nc.dram_tensor("bk", [P, DC], dt.float32, kind="ExternalInput")
    bv = nc.dram_tensor("bv", [HD, H], dt.float32, kind="ExternalInput")
    bproj = nc.dram_tensor("bproj", [P, DC], dt.float32, kind="ExternalInput")
    bfc = nc.dram_tensor("bfc", [P, IC], dt.float32, kind="ExternalInput")
    bmlp = nc.dram_tensor("bmlp", [P, DC], dt.float32, kind="ExternalInput")
    g1 = nc.dram_tensor("g1", [P, DC], dt.float32, kind="ExternalInput")
    be1 = nc.dram_tensor("be1", [P, DC], dt.float32, kind="ExternalInput")
    g2 = nc.dram_tensor("g2", [P, DC], dt.float32, kind="ExternalInput")
    be2 = nc.dram_tensor("be2", [P, DC], dt.float32, kind="ExternalInput")
    outT = nc.dram_tensor("outT", [D, Q], dt.float32, kind="ExternalOutput")

    hT_r = hT.rearrange("(c p) n -> p c n", p=P)
    maskband_r = maskband.rearrange("(k p) n -> p k n", p=P)

    with tile.TileContext(nc) as tc:
        with (
            tc.tile_pool(name="const", bufs=1) as const,
            tc.tile_pool(name="stats", bufs=1) as stats,
            tc.tile_pool(name="tmp", bufs=2) as tmp,
            tc.tile_pool(name="hstream", bufs=3) as hstream,
            tc.tile_pool(name="persist", bufs=1) as persist,
            tc.tile_pool(name="ps", bufs=4, space="PSUM") as ps,
            tc.tile_pool(name="lnps", bufs=2, space="PSUM") as lnps,
        ):
            ones_col = const.tile([P, 1], dt.float32)
            nc.vector.memset(ones_col[:], 1.0)
            ones_row = const.tile([1, P], dt.float32)
            nc.vector.memset(ones_row[:], 1.0)
            ones65 = const.tile([65, HD], dt.float32)
            nc.vector.memset(ones65[:], 1.0)
            eps_t = const.tile([1, 1], dt.float32)
            nc.vector.memset(eps_t[:], EPS)

            def load_pvec(t):
                s = const.tile(list(t.shape), dt.float32, tag=t.name)
                nc.sync.dma_start(s[:], t[:])
                return s

            maskb_s = load_pvec(maskb)
            bq_s, bk_s, bv_s = load_pvec(bq), load_pvec(bk), load_pvec(bv)
            bproj_s, bfc_s, bmlp_s = load_pvec(bproj), load_pvec(bfc), load_pvec(bmlp)
            g1_s, be1_s = load_pvec(g1), load_pvec(be1)
            g2_s, be2_s = load_pvec(g2), load_pvec(be2)

            # LN in transposed layout. get_chunk(c, keep) returns a [P, Q]
            # fp32 AP for chunk c (called for stats pass and apply pass).
            # Column stats via ones-matmuls; mean/rstd broadcast across
            # partitions via PE outer products.
            def layernorm_T(get_chunk, g_s, be_s, odt, out_pool, tag,
                            stats_on_pe=False):
                pss = lnps.tile([1, Q], dt.float32, tag="lnps")
                psq = lnps.tile([1, Q], dt.float32, tag="lnps")
                if stats_on_pe:
                    for c in range(DC):
                        xc = get_chunk(c)
                        sq = tmp.tile([P, Q], dt.float32, tag="sq")
                        nc.vector.tensor_tensor(sq[:], xc, xc, Alu.mult)
                        nc.tensor.matmul(pss[:], ones_col[:], xc,
                                         start=(c == 0), stop=(c == DC - 1))
                        nc.tensor.matmul(psq[:], ones_col[:], sq[:],
                                         start=(c == 0), stop=(c == DC - 1))
                else:
                    acc = tmp.tile([P, Q], dt.float32, tag="lnacc")
                    accq = tmp.tile([P, Q], dt.float32, tag="lnaccq")
                    for c in range(DC):
                        xc = get_chunk(c)
                        if c == 0:
                            nc.vector.tensor_copy(acc[:], xc)
                            nc.vector.tensor_tensor(accq[:], xc, xc, Alu.mult)
                        else:
                            nc.vector.tensor_tensor(acc[:], acc[:], xc, Alu.add)
                            sq = tmp.tile([P, Q], dt.float32, tag="sq")
                            nc.vector.tensor_tensor(sq[:], xc, xc, Alu.mult)
                            nc.vector.tensor_tensor(accq[:], accq[:], sq[:],
                                                    Alu.add)
                    nc.tensor.matmul(pss[:], ones_col[:], acc[:],
                                     start=True, stop=True)
                    nc.tensor.matmul(psq[:], ones_col[:], accq[:],
                                     start=True, stop=True)
                mean = stats.tile([1, Q], dt.float32, tag="mean")
                nc.vector.tensor_scalar_mul(mean[:], pss[:], 1.0 / D)
                msq = stats.tile([1, Q], dt.float32, tag="msq")
                nc.vector.tensor_tensor(msq[:], mean[:], mean[:], Alu.mult)
                var = stats.tile([1, Q], dt.float32, tag="var")
                nc.vector.scalar_tensor_tensor(
                    var[:], psq[:], 1.0 / D, msq[:], Alu.mult, Alu.subtract
                )
                nc.scalar.activation(msq[:], var[:], F.Sqrt, bias=eps_t[:])
                nc.vector.reciprocal(msq[:], msq[:])  # msq now holds rstd
                mb = lnps.tile([P, Q], dt.float32, tag="lnps")
                rb = lnps.tile([P, Q], dt.float32, tag="lnps")
                nc.tensor.matmul(mb[:], ones_row[:], mean[:], start=True, stop=True)
                nc.tensor.matmul(rb[:], ones_row[:], msq[:], start=True, stop=True)
                out = out_pool.tile([P, DC, Q], odt, tag=tag)
                for c in range(DC):
                    xc = get_chunk(c)
                    t1 = tmp.tile([P, Q], dt.float32, tag="lnt1")
                    nc.vector.tensor_tensor(t1[:], xc, mb[:], Alu.subtract)
                    nc.vector.scalar_tensor_tensor(
                        out[:, c, :], t1[:], g_s[:, c : c + 1], rb[:],
                        Alu.mult, Alu.mult,
                    )
                    nc.vector.tensor_scalar_add(
                        out[:, c, :], out[:, c, :], be_s[:, c : c + 1]
                    )
                return out

            def resident_chunks(x_sb):
                return lambda c: x_sb[:, c, :]

            h2 = persist.tile([P, DC, Q], dt.float32, tag="h2")

            with tc.tile_pool(name="attnsc", bufs=1) as attnsc:
                qt = attnsc.tile([P, DC, Q], mdt, tag="qt")
                attn_acc = attnsc.tile([65, H, Q], dt.float32, tag="attn_acc")
                v_sb = attnsc.tile([P, NKT, H * 65], mdt, tag="v")
                vview = v_sb[:].rearrange("p k (h x) -> p k h x", x=65)
                nc.vector.tensor_copy(
                    vview[:, :, :, 64:65],
                    ones_col[:].to_broadcast([P, NKT, H, 1]),
                )
                hq_sb = attnsc.tile([P, DC, Q], dt.float32, tag="hq")
                for c in range(DC):
                    nc.sync.dma_start(hq_sb[:, c, :], hT_r[:, c, 0:Q])

                with (
                    tc.tile_pool(name="quarter", bufs=1) as quarter,
                    tc.tile_pool(name="wkv", bufs=3) as wkv,
                    tc.tile_pool(name="wvp", bufs=1) as wvp,
                    tc.tile_pool(name="expp", bufs=3) as expp,
                    tc.tile_pool(name="pvps", bufs=2, space="PSUM") as pvps,
                ):
                    for q in range(NQT):
                        qsl = slice(q * KQ, (q + 1) * KQ)
                        if q == 0:
                            get_chunk = resident_chunks(hq_sb)
                        else:
                            def get_chunk(c, qsl=qsl):
                                hc = hstream.tile([P, Q], dt.float32, tag="hhc")
                                nc.sync.dma_start(hc[:], hT_r[:, c, qsl])
                                return hc[:]
                        xln = layernorm_T(get_chunk, g1_s, be1_s, mdt,
                                          quarter, "xln", stats_on_pe=(q == 0))

                        if q == 0:
                            for p in range(DC):
                                wq_t = wkv.tile([P, DC, P], mdt, tag="wq")
                                nc.sync.dma_start(wq_t[:], w_q[p])
                                psq_ = ps.tile([P, Q], dt.float32, tag="mm")
                                for c in range(DC):
                                    nc.tensor.matmul(
                                        psq_[:], wq_t[:, c, :], xln[:, c, :],
                                        start=(c == 0), stop=(c == DC - 1),
                                    )
                                nc.scalar.activation(
                                    qt[:, p, :], psq_[:], F.Identity,
                                    bias=bq_s[:, p : p + 1],
                                )

                        kt_sb = quarter.tile([P, DC, KQ], mdt, tag="kt")
                        for p in range(DC):
                            wk_t = wkv.tile([P, DC, P], mdt, tag="wq")
                            nc.sync.dma_start(wk_t[:], w_k[p])
                            psk = ps.tile([P, Q], dt.float32, tag="mm")
                            for c in range(DC):
                                nc.tensor.matmul(
                                    psk[:], wk_t[:, c, :], xln[:, c, :],
                                    start=(c == 0), stop=(c == DC - 1),
                                )
                            nc.scalar.activation(
                                kt_sb[:, p, :], psk[:], F.Identity,
                                bias=bk_s[:, p : p + 1],
                            )

                        for vs in range(2):
                            wv_t = wvp.tile([P, DC, 512], mdt, tag="wv")
                            nc.sync.dma_start(wv_t[:], w_v[vs])
                            for kt in range(NKT):
                                psv = ps.tile([P, Q], dt.float32, tag="mm")
                                for c in range(DC):
                                    nc.tensor.matmul(
                                        psv[:],
                                        xln[:, c, kt * P : (kt + 1) * P],
                                        wv_t[:, c, :],
                                        start=(c == 0), stop=(c == DC - 1),
                                    )
                                dst = v_sb[
                                    :, kt, vs * 8 * 65 : (vs + 1) * 8 * 65
                                ].rearrange("p (h x) -> p h x", x=65)[:, :, 0:64]
                                nc.scalar.activation(
                                    dst,
                                    psv[:].rearrange("p (h x) -> p h x", x=64),
                                    F.Copy,
                                )

                        if q == 0:
                            mask_q = quarter.tile([P, NKT, Q], dt.float32,
                                                  tag="mask")
                            nc.sync.dma_start(mask_q[:], maskband_r[:])

                        for h in range(H):
                            hp, hs = h // 2, (h % 2) * 64
                            pa = pvps.tile([65, Q], dt.float32, tag="pv")
                            for kt in range(NKT):
                                pss = ps.tile([P, Q], dt.float32, tag="mm")
                                nc.tensor.matmul(
                                    pss[:],
                                    kt_sb[hs : hs + 64, hp, kt * P : (kt + 1) * P],
                                    qt[hs : hs + 64, hp, :],
                                    start=True, stop=True,
                                )
                                if q == 0:
                                    nc.vector.tensor_tensor(
                                        pss[:], pss[:], mask_q[:, kt, :], Alu.add
                                    )
                                et = expp.tile([P, Q], mdt, tag="exp")
                                nc.scalar.activation(
                                    et[:], pss[:], F.Exp, scale=0.125,
                                    bias=maskb_s[:, q * NKT + kt : q * NKT + kt + 1],
                                )
                                nc.tensor.matmul(
                                    pa[:], v_sb[:, kt, h * 65 : h * 65 + 65],
                                    et[:],
                                    start=(kt == 0), stop=(kt == NKT - 1),
                                )
                            if q == 0:
                                nc.scalar.activation(
                                    attn_acc[:, h, :], pa[:], F.Copy
                                )
                            else:
                                nc.vector.tensor_tensor(
                                    attn_acc[:, h, :], attn_acc[:, h, :],
                                    pa[:], Alu.add,
                                )

                # normalize per head -> attnT [64, H, Q], then proj as a
                # plain GEMM over the 16 head-chunks + residual -> h2.
                with tc.tile_pool(name="projsc", bufs=1) as projsc, \
                     tc.tile_pool(name="pstream", bufs=2) as pstream:
                    attnT = projsc.tile([HD, H, Q], mdt, tag="attnT")
                    for h in range(H):
                        nc.vector.reciprocal(
                            attn_acc[64:65, h, :], attn_acc[64:65, h, :]
                        )
                        bc = lnps.tile([P, Q], dt.float32, tag="lnps")
                        nc.tensor.matmul(
                            bc[0:64, :], ones65[64:65, :],
                            attn_acc[64:65, h, :], start=True, stop=True,
                        )
                        t1 = tmp.tile([HD, Q], dt.float32, tag="anorm")
                        nc.vector.tensor_tensor(
                            t1[:], attn_acc[0:64, h, :], bc[0:64, :], Alu.mult
                        )
                        nc.vector.tensor_scalar_add(
                            attnT[:, h, :], t1[:], bv_s[:, h : h + 1]
                        )
                    for mo in range(DC):
                        wp_t = pstream.tile([HD, H, P], mdt, tag="wp")
                        nc.sync.dma_start(wp_t[:], w_projr[mo])
                        psp = ps.tile([P, Q], dt.float32, tag="mm")
                        for c in range(H):
                            nc.tensor.matmul(
                                psp[:], wp_t[:, c, :], attnT[:, c, :],
                                start=(c == 0), stop=(c == H - 1),
                            )
                        nc.vector.scalar_tensor_tensor(
                            h2[:, mo, :], psp[:], bproj_s[:, mo : mo + 1],
                            hq_sb[:, mo, :], Alu.add, Alu.add,
                        )

            # ---- LN2 / fc+gelu / mlp + residual ----
            with (
                tc.tile_pool(name="mlpsc", bufs=1) as mlpsc,
                tc.tile_pool(name="wfcs", bufs=3) as wfcs,
                tc.tile_pool(name="wmlps", bufs=3) as wmlps,
            ):
                h2n = layernorm_T(resident_chunks(h2), g2_s, be2_s, mdt,
                                  mlpsc, "h2n", stats_on_pe=True)
                y2 = mlpsc.tile([P, DC, Q], dt.float32, tag="y2")
                g_half = mlpsc.tile([P, IC // 2, Q], mdt, tag="g")
                for ih in range(2):
                    for m in range(IC // 2):
                        mg = ih * (IC // 2) + m
                        wfc_t = wfcs.tile([P, DC, P], mdt, tag="wfc")
                        nc.sync.dma_start(wfc_t[:], w_fcr[mg])
                        psf = ps.tile([P, Q], dt.float32, tag="mm")
                        for c in range(DC):
                            nc.tensor.matmul(
                                psf[:], wfc_t[:, c, :], h2n[:, c, :],
                                start=(c == 0), stop=(c == DC - 1),
                            )
                        nc.scalar.activation(
                            g_half[:, m, :], psf[:], F.Gelu,
                            bias=bfc_s[:, mg : mg + 1],
                        )
                    for mo in range(DC):
                        wm_t = wmlps.tile([P, IC // 2, P], mdt, tag="wmlp")
                        nc.sync.dma_start(wm_t[:], w_mlpr[ih, mo])
                        psm = ps.tile([P, Q], dt.float32, tag="mm")
                        for c in range(IC // 2):
                            nc.tensor.matmul(
                                psm[:], wm_t[:, c, :], g_half[:, c, :],
                                start=(c == 0), stop=(c == IC // 2 - 1),
                            )
                        if ih == 0:
                            nc.scalar.activation(y2[:, mo, :], psm[:], F.Copy)
                        else:
                            ot = tmp.tile([P, Q], dt.float32, tag="outt")
                            nc.vector.tensor_tensor(
                                ot[:], y2[:, mo, :], psm[:], Alu.add
                            )
                            nc.vector.scalar_tensor_tensor(
                                ot[:], ot[:], bmlp_s[:, mo : mo + 1],
                                h2[:, mo, :], Alu.add, Alu.add,
                            )
                            nc.sync.dma_start(
                                outT.rearrange("(c p) n -> p c n", p=P)[:, mo, :],
                                ot[:],
                            )

    nc.compile()
    return nc


def _get_nc(mm_r: bool):
    if mm_r not in _BUILD_CACHE:
        _BUILD_CACHE[mm_r] = _build(mm_r)
    return _BUILD_CACHE[mm_r]


def _prep_in_maps(inputs):
    h = np.asarray(inputs["hidden_states"], dtype=np.float32)
    b_qkv = np.asarray(inputs["b_qkv"], np.float32)
    w_qkv = np.asarray(inputs["w_qkv"], np.float32)

    def chunk_w(w, p=P):  # [Din, N] -> [p, Din//p, N]
        return np.ascontiguousarray(w.reshape(-1, p, w.shape[1]).transpose(1, 0, 2))

    def pvec(v, p=P):  # [n*p] -> [p, n]
        return np.ascontiguousarray(v.reshape(-1, p).T)

    def mslice(a, nsl):  # [p, c, n] -> [n//nsl, p, c, nsl]
        p, c, n = a.shape
        return np.ascontiguousarray(
            a.reshape(p, c, n // nsl, nsl).transpose(2, 0, 1, 3)
        )

    wq = mslice(chunk_w(w_qkv[:, 0:D]), P)
    wk = mslice(chunk_w(w_qkv[:, D : 2 * D]), P)
    wv = mslice(chunk_w(w_qkv[:, 2 * D : 3 * D]), 512)
    w_proj = np.asarray(inputs["w_proj"], np.float32)
    wp = np.ascontiguousarray(
        w_proj.reshape(H, HD, DC, P).transpose(2, 1, 0, 3)
    )
    wfc = mslice(chunk_w(np.asarray(inputs["w_fc"], np.float32)), P)
    wm = chunk_w(np.asarray(inputs["w_mlp"], np.float32))  # [128, 32, 1024]
    wmlp = np.ascontiguousarray(
        wm.reshape(P, 2, IC // 2, DC, P).transpose(1, 3, 0, 2, 4)
    )
    vis = np.arange(Q)[:, None] <= np.arange(Q)[None, :]  # key i visible to query u
    maskband = np.where(vis, np.float32(0.0), np.float32(-10000.0))

    shared = {
        "w_q": wq, "w_k": wk, "w_v": wv, "w_projr": wp,
        "w_fcr": wfc, "w_mlpr": wmlp,
        "maskband": np.ascontiguousarray(maskband.astype(np.float32)),
        "bq": pvec(b_qkv[0:D]),
        "bk": pvec(b_qkv[D : 2 * D]),
        "bv": pvec(b_qkv[2 * D : 3 * D], p=HD),
        "bproj": pvec(np.asarray(inputs["b_proj"], np.float32)),
        "bfc": pvec(np.asarray(inputs["b_fc"], np.float32)),
        "bmlp": pvec(np.asarray(inputs["b_mlp"], np.float32)),
        "g1": pvec(np.asarray(inputs["g1"], np.float32)),
        "be1": pvec(np.asarray(inputs["be1"], np.float32)),
        "g2": pvec(np.asarray(inputs["g2"], np.float32)),
        "be2": pvec(np.asarray(inputs["be2"], np.float32)),
    }
    in_maps = []
    for core in range(8):
        b, j = core // 4, core % 4
        perm = (np.arange(S) + j * Q) % S  # own rows become keys 0..511
        hrot = h[b, perm]
        # per-key exp bias: -10000/8 for keys strictly after the own
        # block (never visible); 0 otherwise (quarter 0 is handled by
        # the triangular band mask).
        masked = perm >= (j + 1) * Q
        mb = np.where(masked, np.float32(-1250.0), np.float32(0.0))
        maskb = np.ascontiguousarray(mb.reshape(NQT * NKT, P).T)
        in_maps.append(
            dict(
                shared,
                hT=np.ascontiguousarray(hrot.T),
                maskb=maskb.astype(np.float32),
            )
        )
    return in_maps


def _stitch(results):
    out = np.empty((2, S, D), dtype=np.float32)
    for core in range(8):
        b, j = core // 4, core % 4
        out[b, j * Q : (j + 1) * Q] = results[core]["outT"].T
    return out


def run(inputs, mm_r=True, trace=False, trace_cores=None):
    nc = _get_nc(mm_r)
    in_maps = _prep_in_maps(inputs)
    res = bass_utils.run_bass_kernel_spmd(
        nc, in_maps, core_ids=list(range(8)), trace=trace, trace_cores=trace_cores
    )
    return _stitch(res.results), res


def kernel(**inputs) -> np.ndarray:
    out, _ = run(inputs, mm_r=True)
    return out


# revision 23
# speedup vs baseline: 1.6868x; 1.2844x over previous
"""GPT2 block kernel for 8 TRN2 NeuronCores (Bass/Tile, SPMD).

Sharding: the 4096 rows (batch*seq) are split 8 ways -> 512 rows/core
(4 cores per batch element). Each core redundantly computes K,V for its
batch, then causal attention for its own 512 query rows against all
2048 keys, then proj/LN2/MLP for its own rows only. Zero collectives.

All tensors are kept in a "transposed" layout (feature dim on SBUF
partitions, token dim on the free axis) so no on-device transposes are
needed. The host transposes inputs/outputs and rotates each core's key
order so its own rows are always key-quarter 0 (attention is
permutation-invariant under a matching mask, which is passed as data).
"""

import numpy as np
import sys

sys.path.insert(0, "/opt/trn_rl_repo")

import concourse.bacc as bacc
import concourse.mybir as mybir
import concourse.tile as tile
from concourse import bass_utils

dt = mybir.dt
F = mybir.ActivationFunctionType
Alu = mybir.AluOpType

D = 1024
S = 2048
Q = 512        # own rows per core
H = 16
HD = 64
INNER = 4096
P = 128
DC = D // P    # 8
IC = INNER // P  # 32
EPS = 1e-5
NQT = 4        # key quarters
KQ = S // NQT  # 512 keys per quarter
NKT = KQ // P  # 4 key tiles per quarter

_BUILD_CACHE = {}


_DT = {"f32": dt.float32, "f32r": dt.float32r, "bf16": dt.bfloat16}


def _build(cfg):
    adt = _DT[cfg[0]]   # attention path: qkv/scores/PV operands
    pdt = _DT[cfg[1]]   # proj/fc/mlp path operands
    nc = bacc.Bacc("TRN2", target_bir_lowering=False, debug=False)

    hT = nc.dram_tensor("hT", [D, S], dt.float32, kind="ExternalInput")
    maskband = nc.dram_tensor("maskband", [Q, Q], dt.float32, kind="ExternalInput")
    maskb = nc.dram_tensor("maskb", [P, NQT * NKT], dt.float32, kind="ExternalInput")
    # weights arrive pre-tiled from the host in the exact consumption
    # order so every weight DMA is fully contiguous on HWDGE
    w_q = nc.dram_tensor("w_q", [DC, P, DC, P], adt, kind="ExternalInput")
    w_k = nc.dram_tensor("w_k", [DC, P, DC, P], adt, kind="ExternalInput")
    w_v = nc.dram_tensor("w_v", [2, P, DC, 512], adt, kind="ExternalInput")
    w_projr = nc.dram_tensor("w_projr", [DC, HD, H, P], pdt, kind="ExternalInput")
    w_fcr = nc.dram_tensor("w_fcr", [IC, P, DC, P], pdt, kind="ExternalInput")
    w_mlpr = nc.dram_tensor("w_mlpr", [2, DC, P, IC // 2, P], pdt, kind="ExternalInput")
    bq = nc.dram_tensor("bq", [P, DC], dt.float32, kind="ExternalInput")
    bk = nc.dram_tensor("bk", [P, DC], dt.float32, kind="ExternalInput")
    bv = nc.dram_tensor("bv", [HD, H], dt.float32, kind="ExternalInput")
    bproj = nc.dram_tensor("bproj", [P, DC], dt.float32, kind="ExternalInput")
    bfc = nc.dram_tensor("bfc", [P, IC], dt.float32, kind="ExternalInput")
    bmlp = nc.dram_tensor("bmlp", [P, DC], dt.float32, kind="ExternalInput")
    g1 = nc.dram_tensor("g1", [P, DC], dt.float32, kind="ExternalInput")
    be1 = nc.dram_tensor("be1", [P, DC], dt.float32, kind="ExternalInput")
    g2 = nc.dram_tensor("g2", [P, DC], dt.float32, kind="ExternalInput")
    be2 = nc.dram_tensor("be2", [P, DC], dt.float32, kind="ExternalInput")
    outT = nc.dram_tensor("outT", [D, Q], dt.float32, kind="ExternalOutput")

    hT_r = hT.rearrange("(c p) n -> p c n", p=P)
    maskband_r = maskband.rearrange("(k p) n -> p k n", p=P)

    with tile.TileContext(nc) as tc:
        with (
            tc.tile_pool(name="const", bufs=1) as const,
            tc.tile_pool(name="stats", bufs=1) as stats,
            tc.tile_pool(name="tmp", bufs=2) as tmp,
            tc.tile_pool(name="hstream", bufs=3) as hstream,
            tc.tile_pool(name="persist", bufs=1) as persist,
            tc.tile_pool(name="ps", bufs=4, space="PSUM") as ps,
            tc.tile_pool(name="lnps", bufs=2, space="PSUM") as lnps,
        ):
            ones_col = const.tile([P, 1], dt.float32)
            nc.vector.memset(ones_col[:], 1.0)
            ones_row = const.tile([1, P], dt.float32)
            nc.vector.memset(ones_row[:], 1.0)
            ones65 = const.tile([65, HD], dt.float32)
            nc.vector.memset(ones65[:], 1.0)
            eps_t = const.tile([1, 1], dt.float32)
            nc.vector.memset(eps_t[:], EPS)

            def load_pvec(t):
                s = const.tile(list(t.shape), dt.float32, tag=t.name)
                nc.sync.dma_start(s[:], t[:])
                return s

            maskb_s = load_pvec(maskb)
            bq_s, bk_s, bv_s = load_pvec(bq), load_pvec(bk), load_pvec(bv)
            bproj_s, bfc_s, bmlp_s = load_pvec(bproj), load_pvec(bfc), load_pvec(bmlp)
            g1_s, be1_s = load_pvec(g1), load_pvec(be1)
            g2_s, be2_s = load_pvec(g2), load_pvec(be2)

            # LN in transposed layout. get_chunk(c, keep) returns a [P, Q]
            # fp32 AP for chunk c (called for stats pass and apply pass).
            # Column stats via ones-matmuls; mean/rstd broadcast across
            # partitions via PE outer products.
            def layernorm_T(get_chunk, g_s, be_s, odt, out_pool, tag,
                            stats_on_pe=False):
                pss = lnps.tile([1, Q], dt.float32, tag="lnps")
                psq = lnps.tile([1, Q], dt.float32, tag="lnps")
                if stats_on_pe:
                    for c in range(DC):
                        xc = get_chunk(c)
                        sq = tmp.tile([P, Q], dt.float32, tag="sq")
                        nc.vector.tensor_tensor(sq[:], xc, xc, Alu.mult)
                        nc.tensor.matmul(pss[:], ones_col[:], xc,
                                         start=(c == 0), stop=(c == DC - 1))
                        nc.tensor.matmul(psq[:], ones_col[:], sq[:],
                                         start=(c == 0), stop=(c == DC - 1))
                else:
                    acc = tmp.tile([P, Q], dt.float32, tag="lnacc")
                    accq = tmp.tile([P, Q], dt.float32, tag="lnaccq")
                    for c in range(DC):
                        xc = get_chunk(c)
                        if c == 0:
                            nc.vector.tensor_copy(acc[:], xc)
                            nc.vector.tensor_tensor(accq[:], xc, xc, Alu.mult)
                        else:
                            nc.vector.tensor_tensor(acc[:], acc[:], xc, Alu.add)
                            sq = tmp.tile([P, Q], dt.float32, tag="sq")
                            nc.vector.tensor_tensor(sq[:], xc, xc, Alu.mult)
                            nc.vector.tensor_tensor(accq[:], accq[:], sq[:],
                                                    Alu.add)
                    nc.tensor.matmul(pss[:], ones_col[:], acc[:],
                                     start=True, stop=True)
                    nc.tensor.matmul(psq[:], ones_col[:], accq[:],
                                     start=True, stop=True)
                mean = stats.tile([1, Q], dt.float32, tag="mean")
                nc.vector.tensor_scalar_mul(mean[:], pss[:], 1.0 / D)
                msq = stats.tile([1, Q], dt.float32, tag="msq")
                nc.vector.tensor_tensor(msq[:], mean[:], mean[:], Alu.mult)
                var = stats.tile([1, Q], dt.float32, tag="var")
                nc.vector.scalar_tensor_tensor(
                    var[:], psq[:], 1.0 / D, msq[:], Alu.mult, Alu.subtract
                )
                nc.scalar.activation(msq[:], var[:], F.Sqrt, bias=eps_t[:])
                nc.vector.reciprocal(msq[:], msq[:])  # msq now holds rstd
                mb = lnps.tile([P, Q], dt.float32, tag="lnps")
                rb = lnps.tile([P, Q], dt.float32, tag="lnps")
                nc.tensor.matmul(mb[:], ones_row[:], mean[:], start=True, stop=True)
                nc.tensor.matmul(rb[:], ones_row[:], msq[:], start=True, stop=True)
                out = out_pool.tile([P, DC, Q], odt, tag=tag)
                for c in range(DC):
                    xc = get_chunk(c)
                    t1 = tmp.tile([P, Q], dt.float32, tag="lnt1")
                    nc.vector.tensor_tensor(t1[:], xc, mb[:], Alu.subtract)
                    nc.vector.scalar_tensor_tensor(
                        out[:, c, :], t1[:], g_s[:, c : c + 1], rb[:],
                        Alu.mult, Alu.mult,
                    )
                    nc.vector.tensor_scalar_add(
                        out[:, c, :], out[:, c, :], be_s[:, c : c + 1]
                    )
                return out

            def resident_chunks(x_sb):
                return lambda c: x_sb[:, c, :]

            h2 = persist.tile([P, DC, Q], dt.float32, tag="h2")

            with tc.tile_pool(name="attnsc", bufs=1) as attnsc:
                qt = attnsc.tile([P, DC, Q], adt, tag="qt")
                attn_acc = attnsc.tile([65, H, Q], dt.float32, tag="attn_acc")
                v_sb = attnsc.tile([P, NKT, H * 65], adt, tag="v")
                vview = v_sb[:].rearrange("p k (h x) -> p k h x", x=65)
                nc.vector.tensor_copy(
                    vview[:, :, :, 64:65],
                    ones_col[:].to_broadcast([P, NKT, H, 1]),
                )
                hq_sb = attnsc.tile([P, DC, Q], dt.float32, tag="hq")
                for c in range(DC):
                    nc.sync.dma_start(hq_sb[:, c, :], hT_r[:, c, 0:Q])

                with (
                    tc.tile_pool(name="quarter", bufs=1) as quarter,
                    tc.tile_pool(name="wkv", bufs=3) as wkv,
                    tc.tile_pool(name="wvp", bufs=1) as wvp,
                    tc.tile_pool(name="expp", bufs=3) as expp,
                    tc.tile_pool(name="pvps", bufs=2, space="PSUM") as pvps,
                ):
                    for q in range(NQT):
                        qsl = slice(q * KQ, (q + 1) * KQ)
                        if q == 0:
                            get_chunk = resident_chunks(hq_sb)
                        else:
                            def get_chunk(c, qsl=qsl):
                                hc = hstream.tile([P, Q], dt.float32, tag="hhc")
                                nc.sync.dma_start(hc[:], hT_r[:, c, qsl])
                                return hc[:]
                        xln = layernorm_T(get_chunk, g1_s, be1_s, adt,
                                          quarter, "xln", stats_on_pe=(q == 0))

                        if q == 0:
                            for p in range(DC):
                                wq_t = wkv.tile([P, DC, P], adt, tag="wq")
                                nc.sync.dma_start(wq_t[:], w_q[p])
                                psq_ = ps.tile([P, Q], dt.float32, tag="mm")
                                for c in range(DC):
                                    nc.tensor.matmul(
                                        psq_[:], wq_t[:, c, :], xln[:, c, :],
                                        start=(c == 0), stop=(c == DC - 1),
                                    )
                                nc.scalar.activation(
                                    qt[:, p, :], psq_[:], F.Identity,
                                    bias=bq_s[:, p : p + 1],
                                )

                        kt_sb = quarter.tile([P, DC, KQ], adt, tag="kt")
                        for p in range(DC):
                            wk_t = wkv.tile([P, DC, P], adt, tag="wq")
                            nc.sync.dma_start(wk_t[:], w_k[p])
                            psk = ps.tile([P, Q], dt.float32, tag="mm")
                            for c in range(DC):
                                nc.tensor.matmul(
                                    psk[:], wk_t[:, c, :], xln[:, c, :],
                                    start=(c == 0), stop=(c == DC - 1),
                                )
                            nc.scalar.activation(
                                kt_sb[:, p, :], psk[:], F.Identity,
                                bias=bk_s[:, p : p + 1],
                            )

                        for vs in range(2):
                            wv_t = wvp.tile([P, DC, 512], adt, tag="wv")
                            nc.sync.dma_start(wv_t[:], w_v[vs])
                            for kt in range(NKT):
                                psv = ps.tile([P, Q], dt.float32, tag="mm")
                                for c in range(DC):
                                    nc.tensor.matmul(
                                        psv[:],
                                        xln[:, c, kt * P : (kt + 1) * P],
                                        wv_t[:, c, :],
                                        start=(c == 0), stop=(c == DC - 1),
                                    )
                                dst = v_sb[
                                    :, kt, vs * 8 * 65 : (vs + 1) * 8 * 65
                                ].rearrange("p (h x) -> p h x", x=65)[:, :, 0:64]
                                nc.scalar.activation(
                                    dst,
                                    psv[:].rearrange("p (h x) -> p h x", x=64),
                                    F.Copy,
                                )

                        if q == 0:
                            mask_q = quarter.tile([P, NKT, Q], dt.float32,
                                                  tag="mask")
                            nc.sync.dma_start(mask_q[:], maskband_r[:])

                        for h in range(H):
                            hp, hs = h // 2, (h % 2) * 64
                            pa = pvps.tile([65, Q], dt.float32, tag="pv")
                            for kt in range(NKT):
                                pss = ps.tile([P, Q], dt.float32, tag="mm")
                                nc.tensor.matmul(
                                    pss[:],
                                    kt_sb[hs : hs + 64, hp, kt * P : (kt + 1) * P],
                                    qt[hs : hs + 64, hp, :],
                                    start=True, stop=True,
                                )
                                if q == 0:
                                    nc.vector.tensor_tensor(
                                        pss[:], pss[:], mask_q[:, kt, :], Alu.add
                                    )
                                et = expp.tile([P, Q], adt, tag="exp")
                                nc.scalar.activation(
                                    et[:], pss[:], F.Exp, scale=0.125,
                                    bias=maskb_s[:, q * NKT + kt : q * NKT + kt + 1],
                                )
                                nc.tensor.matmul(
                                    pa[:], v_sb[:, kt, h * 65 : h * 65 + 65],
                                    et[:],
                                    start=(kt == 0), stop=(kt == NKT - 1),
                                )
                            if q == 0:
                                nc.scalar.activation(
                                    attn_acc[:, h, :], pa[:], F.Copy
                                )
                            else:
                                nc.vector.tensor_tensor(
                                    attn_acc[:, h, :], attn_acc[:, h, :],
                                    pa[:], Alu.add,
                                )

                # normalize per head -> attnT [64, H, Q], then proj as a
                # plain GEMM over the 16 head-chunks + residual -> h2.
                with tc.tile_pool(name="projsc", bufs=1) as projsc, \
                     tc.tile_pool(name="pstream", bufs=2) as pstream:
                    attnT = projsc.tile([HD, H, Q], pdt, tag="attnT")
                    for h in range(H):
                        nc.vector.reciprocal(
                            attn_acc[64:65, h, :], attn_acc[64:65, h, :]
                        )
                        bc = lnps.tile([P, Q], dt.float32, tag="lnps")
                        nc.tensor.matmul(
                            bc[0:64, :], ones65[64:65, :],
                            attn_acc[64:65, h, :], start=True, stop=True,
                        )
                        t1 = tmp.tile([HD, Q], dt.float32, tag="anorm")
                        nc.vector.tensor_tensor(
                            t1[:], attn_acc[0:64, h, :], bc[0:64, :], Alu.mult
                        )
                        nc.vector.tensor_scalar_add(
                            attnT[:, h, :], t1[:], bv_s[:, h : h + 1]
                        )
                    for mo in range(DC):
                        wp_t = pstream.tile([HD, H, P], pdt, tag="wp")
                        nc.sync.dma_start(wp_t[:], w_projr[mo])
                        psp = ps.tile([P, Q], dt.float32, tag="mm")
                        for c in range(H):
                            nc.tensor.matmul(
                                psp[:], wp_t[:, c, :], attnT[:, c, :],
                                start=(c == 0), stop=(c == H - 1),
                            )
                        nc.vector.scalar_tensor_tensor(
                            h2[:, mo, :], psp[:], bproj_s[:, mo : mo + 1],
                            hq_sb[:, mo, :], Alu.add, Alu.add,
                        )

            # ---- LN2 / fc+gelu / mlp + residual ----
            with (
                tc.tile_pool(name="mlpsc", bufs=1) as mlpsc,
                tc.tile_pool(name="wfcs", bufs=3) as wfcs,
                tc.tile_pool(name="wmlps", bufs=3) as wmlps,
            ):
                h2n = layernorm_T(resident_chunks(h2), g2_s, be2_s, pdt,
                                  mlpsc, "h2n", stats_on_pe=True)
                y2 = mlpsc.tile([P, DC, Q], dt.float32, tag="y2")
                g_half = mlpsc.tile([P, IC // 2, Q], pdt, tag="g")
                for ih in range(2):
                    for m in range(IC // 2):
                        mg = ih * (IC // 2) + m
                        wfc_t = wfcs.tile([P, DC, P], pdt, tag="wfc")
                        nc.sync.dma_start(wfc_t[:], w_fcr[mg])
                        psf = ps.tile([P, Q], dt.float32, tag="mm")
                        for c in range(DC):
                            nc.tensor.matmul(
                                psf[:], wfc_t[:, c, :], h2n[:, c, :],
                                start=(c == 0), stop=(c == DC - 1),
                            )
                        nc.scalar.activation(
                            g_half[:, m, :], psf[:], F.Gelu,
                            bias=bfc_s[:, mg : mg + 1],
                        )
                    for mo in range(DC):
                        wm_t = wmlps.tile([P, IC // 2, P], pdt, tag="wmlp")
                        nc.sync.dma_start(wm_t[:], w_mlpr[ih, mo])
                        psm = ps.tile([P, Q], dt.float32, tag="mm")
                        for c in range(IC // 2):
                            nc.tensor.matmul(
                                psm[:], wm_t[:, c, :], g_half[:, c, :],
                                start=(c == 0), stop=(c == IC // 2 - 1),
                            )
                        if ih == 0:
                            nc.scalar.activation(y2[:, mo, :], psm[:], F.Copy)
                        else:
                            ot = tmp.tile([P, Q], dt.float32, tag="outt")
                            nc.vector.tensor_tensor(
                                ot[:], y2[:, mo, :], psm[:], Alu.add
                            )
                            nc.vector.scalar_tensor_tensor(
                                ot[:], ot[:], bmlp_s[:, mo : mo + 1],
                                h2[:, mo, :], Alu.add, Alu.add,
                            )
                            nc.sync.dma_start(
                                outT.rearrange("(c p) n -> p c n", p=P)[:, mo, :],
                                ot[:],
                            )

    nc.compile()
    return nc


def _get_nc(cfg):
    if cfg not in _BUILD_CACHE:
        _BUILD_CACHE[cfg] = _build(cfg)
    return _BUILD_CACHE[cfg]


def _np_dt(name):
    if name == "bf16":
        import ml_dtypes
        return ml_dtypes.bfloat16
    return np.float32


def _prep_in_maps(inputs, cfg):
    adt_np, pdt_np = _np_dt(cfg[0]), _np_dt(cfg[1])
    h = np.asarray(inputs["hidden_states"], dtype=np.float32)
    b_qkv = np.asarray(inputs["b_qkv"], np.float32)
    w_qkv = np.asarray(inputs["w_qkv"], np.float32)

    def chunk_w(w, p=P):  # [Din, N] -> [p, Din//p, N]
        return np.ascontiguousarray(w.reshape(-1, p, w.shape[1]).transpose(1, 0, 2))

    def pvec(v, p=P):  # [n*p] -> [p, n]
        return np.ascontiguousarray(v.reshape(-1, p).T)

    def mslice(a, nsl):  # [p, c, n] -> [n//nsl, p, c, nsl]
        p, c, n = a.shape
        return np.ascontiguousarray(
            a.reshape(p, c, n // nsl, nsl).transpose(2, 0, 1, 3)
        )

    wq = mslice(chunk_w(w_qkv[:, 0:D]), P)
    wk = mslice(chunk_w(w_qkv[:, D : 2 * D]), P)
    wv = mslice(chunk_w(w_qkv[:, 2 * D : 3 * D]), 512)
    w_proj = np.asarray(inputs["w_proj"], np.float32)
    wp = np.ascontiguousarray(
        w_proj.reshape(H, HD, DC, P).transpose(2, 1, 0, 3)
    )
    wfc = mslice(chunk_w(np.asarray(inputs["w_fc"], np.float32)), P)
    wm = chunk_w(np.asarray(inputs["w_mlp"], np.float32))  # [128, 32, 1024]
    wmlp = np.ascontiguousarray(
        wm.reshape(P, 2, IC // 2, DC, P).transpose(1, 3, 0, 2, 4)
    )
    vis = np.arange(Q)[:, None] <= np.arange(Q)[None, :]  # key i visible to query u
    maskband = np.where(vis, np.float32(0.0), np.float32(-10000.0))

    shared = {
        "w_q": wq.astype(adt_np), "w_k": wk.astype(adt_np),
        "w_v": wv.astype(adt_np), "w_projr": wp.astype(pdt_np),
        "w_fcr": wfc.astype(pdt_np), "w_mlpr": wmlp.astype(pdt_np),
        "maskband": np.ascontiguousarray(maskband.astype(np.float32)),
        "bq": pvec(b_qkv[0:D]),
        "bk": pvec(b_qkv[D : 2 * D]),
        "bv": pvec(b_qkv[2 * D : 3 * D], p=HD),
        "bproj": pvec(np.asarray(inputs["b_proj"], np.float32)),
        "bfc": pvec(np.asarray(inputs["b_fc"], np.float32)),
        "bmlp": pvec(np.asarray(inputs["b_mlp"], np.float32)),
        "g1": pvec(np.asarray(inputs["g1"], np.float32)),
        "be1": pvec(np.asarray(inputs["be1"], np.float32)),
        "g2": pvec(np.asarray(inputs["g2"], np.float32)),
        "be2": pvec(np.asarray(inputs["be2"], np.float32)),
    }
    in_maps = []
    for core in range(8):
        b, j = core // 4, core % 4
        perm = (np.arange(S) + j * Q) % S  # own rows become keys 0..511
        hrot = h[b, perm]
        # per-key exp bias: -10000/8 for keys strictly after the own
        # block (never visible); 0 otherwise (quarter 0 is handled by
        # the triangular band mask).
        masked = perm >= (j + 1) * Q
        mb = np.where(masked, np.float32(-1250.0), np.float32(0.0))
        maskb = np.ascontiguousarray(mb.reshape(NQT * NKT, P).T)
        in_maps.append(
            dict(
                shared,
                hT=np.ascontiguousarray(hrot.T),
                maskb=maskb.astype(np.float32),
            )
        )
    return in_maps


def _stitch(results):
    out = np.empty((2, S, D), dtype=np.float32)
    for core in range(8):
        b, j = core // 4, core % 4
        out[b, j * Q : (j + 1) * Q] = results[core]["outT"].T
    return out


def run(inputs, cfg=("bf16", "bf16"), trace=False, trace_cores=None):
    nc = _get_nc(cfg)
    in_maps = _prep_in_maps(inputs, cfg)
    res = bass_utils.run_bass_kernel_spmd(
        nc, in_maps, core_ids=list(range(8)), trace=trace, trace_cores=trace_cores
    )
    return _stitch(res.results), res


def kernel(**inputs) -> np.ndarray:
    out, _ = run(inputs, cfg=("bf16", "bf16"))
    return out
